# revision 11
# baseline (speedup 1.0000x reference)
"""Trainium2 Bass kernel for nn_CrossAttention (efficient-attention form).

Reference computation per batch b:
    K = softmax(x2, axis=-1)           # over D
    Q = softmax(x2, axis=1)            # over N
    out = ((x @ K.T) @ Q) @ W.T + b

Reassociated (matmuls are associative; both softmaxes share exp(x2)):
    E  = exp(x2)                       # one exp serves both softmaxes
    K  = E * (1/rowsum(E))             # per-row scale
    M  = E^T K                         # [D, D]
    s  = 1/colsum(E)  (rowsum of M chunks, since rows of K sum to 1)
    C  = M^T diag(s) W^T               # [D, D]
    out = x @ C + b                    # single [N,D]@[D,D] matmul on x

Schedule: W streams first as a flat 8KB-per-partition load (full DMA
rate; 2KB-row loads run at half rate) and its 16 PE transposes fill the
tensor-engine ramp before the first E/K matmul, writing W^T back with
one stride-4 batched copy per column chunk.  All x2 tiles follow (the
E/K chain paces on them; rowsums alternate ACT-accumulator / DVE
reduce), then all x tiles.  The out phase casts x to bf16 on ACT,
PE-transposes each 128-column block, accumulates x @ C, adds bias on
DVE, and writes one 4KB-contiguous output DMA per row group.
"""

import sys

import numpy as np

if "/opt/trn_rl_repo" not in sys.path:
    sys.path.insert(0, "/opt/trn_rl_repo")

import concourse.bass as bass
import concourse.bass_utils as bass_utils
import concourse.mybir as mybir
import concourse.tile as tile
from concourse import bacc
from concourse.bass import ds, ts
from concourse.bass_utils import run_bass_kernel_spmd
from concourse.masks import make_identity

B, N, D = 8, 2048, 512
P = 128
T = 2             # rows per partition per group
G = N // (P * T)  # 8 row groups
DC = D // P       # 4 column chunks of D
R4 = 4            # W rows per partition in the flat load
F32 = mybir.dt.float32
BF16 = mybir.dt.bfloat16

_CACHE = {}


def _build_nc():
    nc = bacc.Bacc("TRN2", target_bir_lowering=False, debug=False)
    x_d = nc.declare_dram_parameter("x", [N, D], F32, isOutput=False)
    x2_d = nc.declare_dram_parameter("x2", [N, D], F32, isOutput=False)
    w_d = nc.declare_dram_parameter("W", [D, D], F32, isOutput=False)
    b_d = nc.declare_dram_parameter("b", [D], F32, isOutput=False)
    out_d = nc.declare_dram_parameter("out", [N, D], F32, isOutput=True)

    # row n = g*256 + p*2 + t  -> per-partition DRAM span is 2 rows = 4KB
    x_t = x_d[:].rearrange("(g p t) d -> g p t d", p=P, t=T)
    x2_t = x2_d[:].rearrange("(g p t) d -> g p t d", p=P, t=T)
    out_t = out_d[:].rearrange("(g p t) d -> g p t d", p=P, t=T)
    # flat W: partition p holds rows 4p..4p+3 (8KB contiguous, full DMA rate)
    w_f = w_d[:].rearrange("(p r) d -> p r d", p=P)

    with tile.TileContext(nc) as tc:
        with (
            tc.tile_pool(name="big", bufs=1) as big,
            tc.tile_pool(name="x2st", bufs=8) as x2st,
            tc.tile_pool(name="xst", bufs=6) as xst,
            tc.tile_pool(name="xbp", bufs=3) as xbp,
            tc.tile_pool(name="xtt", bufs=3) as xttp,
            tc.tile_pool(name="ogp", bufs=3) as ogp,
            tc.tile_pool(name="small", bufs=1) as small,
            tc.tile_pool(name="stats", bufs=10) as stats,
            tc.tile_pool(name="psM", bufs=1, space="PSUM") as psM,
            tc.tile_pool(name="psT", bufs=2, space="PSUM") as psT,
            tc.tile_pool(name="psO", bufs=2, space="PSUM") as psO,
        ):
            # ---- persistent SBUF tensors
            e_all = big.tile([P, G, T, D], BF16, tag="e_all")    # exp(x2)
            k_all = big.tile([P, G, T, D], BF16, tag="k_all")    # K rows
            wf_all = big.tile([P, R4, D], F32, tag="wf_all")     # W flat
            wt_all = big.tile([P, DC, D], BF16, tag="wt_all")    # W^T
            v_all = big.tile([P, DC, D], BF16, tag="v_all")      # diag(s) W^T
            mt_all = big.tile([P, DC, D], BF16, tag="mt_all")    # (E^T K) rows
            c_all = big.tile([P, DC, D], BF16, tag="c_all")      # C rows
            ident = small.tile([P, P], F32, tag="ident")
            identb = small.tile([P, P], BF16, tag="identb")
            bias_bc = small.tile([P, D], F32, tag="bias_bc")
            # stride-4 view of wt_all for the flat-W transpose writeback
            wt_r = wt_all.rearrange("p j (e r) -> p j r e", r=R4)

            make_identity(nc, ident)
            make_identity(nc, identb)
            b_ap = b_d[:]
            nc.gpsimd.dma_start(
                out=bias_bc,
                in_=bass.AP(tensor=b_ap.tensor, offset=b_ap.offset,
                            ap=[[0, P]] + list(b_ap.ap)),
            )

            # ---- input stream, all on the sync HWDGE queue: x2 first
            # (the ACT exp chain paces on it), W next (its transposes run
            # in the PE-idle normalize window), then every x tile.
            x2_tiles = []
            for g in range(G):
                x2_s = x2st.tile([P, T, D], F32, tag="x2_s")
                nc.sync.dma_start(out=x2_s, in_=x2_t[g])
                x2_tiles.append(x2_s)
            nc.sync.dma_start(out=wf_all, in_=w_f)
            x_tiles = []
            for g in range(G):
                x_s = xst.tile([P, T, D], F32, tag="x_s")
                nc.sync.dma_start(out=x_s, in_=x_t[g])
                x_tiles.append(x_s)

            # ---- phase 1: exp (ACT; rowsum alternates ACT accumulator /
            # DVE reduce), K scale on DVE, E/K matmul chain on PE.
            ps_m = psM.tile([P, DC, D], F32, tag="ps_m")
            for g in range(G):
                x2_s = x2_tiles[g]
                for t in range(T):
                    e_i = e_all[:, g, t, :]
                    rs = stats.tile([P, 1], F32, tag="rs")
                    if t == 0:
                        nc.scalar.activation(
                            out=e_i, in_=x2_s[:, t, :],
                            func=mybir.ActivationFunctionType.Exp,
                            accum_out=rs,
                        )
                    else:
                        nc.scalar.activation(
                            out=e_i, in_=x2_s[:, t, :],
                            func=mybir.ActivationFunctionType.Exp,
                        )
                        nc.vector.tensor_reduce(
                            out=rs, in_=e_i,
                            axis=mybir.AxisListType.X, op=mybir.AluOpType.add)
                    rr = stats.tile([P, 1], F32, tag="rr")
                    nc.vector.reciprocal(out=rr, in_=rs)
                    nc.vector.tensor_scalar_mul(
                        k_all[:, g, t, :], e_i, rr)
                for t in range(T):
                    e_i = e_all[:, g, t, :]
                    k_i = k_all[:, g, t, :]
                    for j in range(DC):
                        nc.tensor.matmul(
                            ps_m[:, j, :],
                            lhsT=e_i[ts(0, P), ts(j, P)],
                            rhs=k_i,
                            start=(g == 0 and t == 0),
                            stop=(g == G - 1 and t == T - 1),
                        )

            # ---- W^T transposes of the flat layout (rows {4p+r} land
            # e-columns stride-4 via the wt_r view); one batched DVE copy
            # per column chunk.  These fill the PE-idle normalize window.
            for kw in range(DC):
                pw = psT.tile([P, R4, P], F32, tag="pt")
                for r in range(R4):
                    nc.tensor.transpose(
                        pw[:, r, :], wf_all[:, r, ts(kw, P)], ident)
                nc.vector.tensor_copy(wt_r[:, kw, :, :], pw)

            # ---- normalize: mt = M rows (bf16), cs_j = rowsum -> s = 1/cs,
            # v = diag(s) W^T.  Chunks alternate DVE / ACT.
            sjs = []
            for j in range(DC):
                cs = stats.tile([P, 1], F32, tag="cs")
                if j % 2 == 0:
                    nc.vector.tensor_scalar(
                        out=mt_all[:, j, :], in0=ps_m[:, j, :],
                        scalar1=1.0, scalar2=0.0,
                        op0=mybir.AluOpType.mult,
                        op1=mybir.AluOpType.add,
                        accum_out=cs,
                    )
                else:
                    nc.scalar.activation(
                        out=mt_all[:, j, :], in_=ps_m[:, j, :],
                        func=mybir.ActivationFunctionType.Copy,
                        accum_out=cs,
                    )
                sj = stats.tile([P, 1], F32, tag="sj")
                nc.vector.reciprocal(out=sj, in_=cs)
                sjs.append(sj)
            for j in range(DC):
                if j % 2 == 0:
                    nc.vector.tensor_scalar_mul(
                        v_all[:, j, :], wt_all[:, j, :], sjs[j])
                else:
                    nc.scalar.mul(v_all[:, j, :], wt_all[:, j, :], sjs[j])

            # ---- C = M^T diag(s) W^T  ([D, D], bf16 chain)
            for k in range(DC):
                pc = psO.tile([P, D], F32, tag="po")
                for j in range(DC):
                    nc.tensor.matmul(
                        pc,
                        lhsT=mt_all[:, j, ts(k, P)],
                        rhs=v_all[:, j, :],
                        start=(j == 0), stop=(j == DC - 1),
                    )
                nc.scalar.copy(c_all[:, k, :], pc)

            # ---- phase 2: cast x -> bf16 (ACT), transpose 128-col blocks
            # (PE, bf16), out matmuls, bias on DVE, one 4KB-contiguous
            # output DMA per row group on the sync queue.
            for g in range(G):
                x_s = x_tiles[g]
                xb = xbp.tile([P, T, D], BF16, tag="xb")
                nc.scalar.copy(xb, x_s)
                og = ogp.tile([P, T, D], F32, tag="og")
                for t in range(T):
                    pt = psT.tile([P, DC, P], BF16, tag="pt")
                    for j in range(DC):
                        nc.tensor.transpose(
                            pt[:, j, :], xb[:, t, ts(j, P)], identb)
                    xt = xttp.tile([P, DC, P], BF16, tag="xt")
                    if t == 0:
                        nc.vector.tensor_copy(xt, pt)
                    else:
                        nc.scalar.copy(xt, pt)
                    po = psO.tile([P, D], F32, tag="po")
                    for j in range(DC):
                        nc.tensor.matmul(
                            po,
                            lhsT=xt[:, j, :],
                            rhs=c_all[:, j, :],
                            start=(j == 0), stop=(j == DC - 1),
                        )
                    nc.vector.tensor_add(og[:, t, :], po, bias_bc)
                nc.sync.dma_start(out=out_t[g], in_=og)

    nc.compile()
    return nc


def get_nc():
    if "nc" not in _CACHE:
        _CACHE["nc"] = _build_nc()
    return _CACHE["nc"]


def kernel(x, x2, W, b, _trace=False):
    nc = get_nc()
    in_maps = [
        {
            "x": np.ascontiguousarray(x[i], dtype=np.float32),
            "x2": np.ascontiguousarray(x2[i], dtype=np.float32),
            "W": np.ascontiguousarray(W, dtype=np.float32),
            "b": np.ascontiguousarray(b, dtype=np.float32),
        }
        for i in range(B)
    ]
    res = run_bass_kernel_spmd(nc, in_maps, list(range(B)), trace=_trace)
    out = np.stack([res.results[i]["out"] for i in range(B)], axis=0)
    if _trace:
        _CACHE["last_results"] = res
    return out


# revision 12
# speedup vs baseline: 1.0376x; 1.0376x over previous
"""Trainium2 Bass kernel for nn_CrossAttention (efficient-attention form).

Reference computation per batch b:
    K = softmax(x2, axis=-1)           # over D
    Q = softmax(x2, axis=1)            # over N
    out = ((x @ K.T) @ Q) @ W.T + b

Reassociated (matmuls are associative; both softmaxes share exp(x2)):
    E  = exp(x2)                       # one exp serves both softmaxes
    K  = E * (1/rowsum(E))             # per-row scale
    M  = E^T K                         # [D, D]
    s  = 1/colsum(E)  (rowsum of M chunks, since rows of K sum to 1)
    C  = M^T diag(s) W^T               # [D, D]
    out = x @ C + b                    # single [N,D]@[D,D] matmul on x

Schedule: W streams first as a flat 8KB-per-partition load (full DMA
rate; 2KB-row loads run at half rate) and its 16 PE transposes fill the
tensor-engine ramp before the first E/K matmul, writing W^T back with
one stride-4 batched copy per column chunk.  All x2 tiles follow (the
E/K chain paces on them; rowsums alternate ACT-accumulator / DVE
reduce), then all x tiles.  The out phase casts x to bf16 on ACT,
PE-transposes each 128-column block, accumulates x @ C, adds bias on
DVE, and writes one 4KB-contiguous output DMA per row group.
"""

import sys

import numpy as np

if "/opt/trn_rl_repo" not in sys.path:
    sys.path.insert(0, "/opt/trn_rl_repo")

import concourse.bass as bass
import concourse.bass_utils as bass_utils
import concourse.mybir as mybir
import concourse.tile as tile
from concourse import bacc
from concourse.bass import ds, ts
from concourse.bass_utils import run_bass_kernel_spmd
from concourse.masks import make_identity

B, N, D = 8, 2048, 512
P = 128
T = 2             # rows per partition per group
G = N // (P * T)  # 8 row groups
DC = D // P       # 4 column chunks of D
R4 = 4            # W rows per partition in the flat load
F32 = mybir.dt.float32
BF16 = mybir.dt.bfloat16

_CACHE = {}


def _build_nc():
    nc = bacc.Bacc("TRN2", target_bir_lowering=False, debug=False)
    x_d = nc.declare_dram_parameter("x", [N, D], F32, isOutput=False)
    x2_d = nc.declare_dram_parameter("x2", [N, D], F32, isOutput=False)
    w_d = nc.declare_dram_parameter("W", [D, D], F32, isOutput=False)
    b_d = nc.declare_dram_parameter("b", [D], F32, isOutput=False)
    out_d = nc.declare_dram_parameter("out", [N, D], F32, isOutput=True)

    # row n = g*256 + p*2 + t  -> per-partition DRAM span is 2 rows = 4KB
    x_t = x_d[:].rearrange("(g p t) d -> g p t d", p=P, t=T)
    x2_t = x2_d[:].rearrange("(g p t) d -> g p t d", p=P, t=T)
    out_t = out_d[:].rearrange("(g p t) d -> g p t d", p=P, t=T)
    # flat W: partition p holds rows 4p..4p+3 (8KB contiguous, full DMA rate)
    w_f = w_d[:].rearrange("(p r) d -> p r d", p=P)

    with tile.TileContext(nc) as tc:
        with (
            tc.tile_pool(name="big", bufs=1) as big,
            tc.tile_pool(name="x2st", bufs=8) as x2st,
            tc.tile_pool(name="xst", bufs=6) as xst,
            tc.tile_pool(name="xbp", bufs=3) as xbp,
            tc.tile_pool(name="xtt", bufs=3) as xttp,
            tc.tile_pool(name="ogp", bufs=3) as ogp,
            tc.tile_pool(name="small", bufs=1) as small,
            tc.tile_pool(name="stats", bufs=10) as stats,
            tc.tile_pool(name="psM", bufs=1, space="PSUM") as psM,
            tc.tile_pool(name="psT", bufs=2, space="PSUM") as psT,
            tc.tile_pool(name="psO", bufs=2, space="PSUM") as psO,
        ):
            # ---- persistent SBUF tensors
            e_all = big.tile([P, G, T, D], BF16, tag="e_all")    # exp(x2)
            k_all = big.tile([P, G, T, D], BF16, tag="k_all")    # K rows
            wf_all = big.tile([P, R4, D], F32, tag="wf_all")     # W flat
            wt_all = big.tile([P, DC, D], BF16, tag="wt_all")    # W^T
            v_all = big.tile([P, DC, D], BF16, tag="v_all")      # diag(s) W^T
            mt_all = big.tile([P, DC, D], BF16, tag="mt_all")    # (E^T K) rows
            c_all = big.tile([P, DC, D], BF16, tag="c_all")      # C rows
            ident = small.tile([P, P], F32, tag="ident")
            identb = small.tile([P, P], BF16, tag="identb")
            bias_bc = small.tile([P, D], F32, tag="bias_bc")
            # stride-4 view of wt_all for the flat-W transpose writeback
            wt_r = wt_all.rearrange("p j (e r) -> p j r e", r=R4)

            make_identity(nc, ident)
            make_identity(nc, identb)
            b_ap = b_d[:]
            nc.gpsimd.dma_start(
                out=bias_bc,
                in_=bass.AP(tensor=b_ap.tensor, offset=b_ap.offset,
                            ap=[[0, P]] + list(b_ap.ap)),
            )

            # ---- input stream, all on the sync HWDGE queue:
            # W first (fills the PE ramp with W^T transposes), then every
            # x2 tile (paces the E/K chain), then every x tile.
            nc.sync.dma_start(out=wf_all, in_=w_f)
            x2_tiles = []
            for g in range(G):
                x2_s = x2st.tile([P, T, D], F32, tag="x2_s")
                nc.sync.dma_start(out=x2_s, in_=x2_t[g])
                x2_tiles.append(x2_s)
            x_tiles = []
            for g in range(G):
                x_s = xst.tile([P, T, D], F32, tag="x_s")
                nc.sync.dma_start(out=x_s, in_=x_t[g])
                x_tiles.append(x_s)

            # ---- W^T transposes of the flat layout (rows {4p+r} land
            # e-columns stride-4 via the wt_r view); one batched DVE copy
            # per column chunk.
            for kw in range(DC):
                pw = psT.tile([P, R4, P], F32, tag="pt")
                for r in range(R4):
                    nc.tensor.transpose(
                        pw[:, r, :], wf_all[:, r, ts(kw, P)], ident)
                nc.vector.tensor_copy(wt_r[:, kw, :, :], pw)

            # ---- phase 1: exp (ACT; rowsum alternates ACT accumulator /
            # DVE reduce), K scale on DVE, E/K matmul chain on PE.
            ps_m = psM.tile([P, DC, D], F32, tag="ps_m")
            for g in range(G):
                x2_s = x2_tiles[g]
                for t in range(T):
                    e_i = e_all[:, g, t, :]
                    rs = stats.tile([P, 1], F32, tag="rs")
                    if t == 0:
                        nc.scalar.activation(
                            out=e_i, in_=x2_s[:, t, :],
                            func=mybir.ActivationFunctionType.Exp,
                            accum_out=rs,
                        )
                    else:
                        nc.scalar.activation(
                            out=e_i, in_=x2_s[:, t, :],
                            func=mybir.ActivationFunctionType.Exp,
                        )
                        nc.vector.tensor_reduce(
                            out=rs, in_=e_i,
                            axis=mybir.AxisListType.X, op=mybir.AluOpType.add)
                    rr = stats.tile([P, 1], F32, tag="rr")
                    nc.vector.reciprocal(out=rr, in_=rs)
                    nc.vector.tensor_scalar_mul(
                        k_all[:, g, t, :], e_i, rr)
                for t in range(T):
                    e_i = e_all[:, g, t, :]
                    k_i = k_all[:, g, t, :]
                    for j in range(DC):
                        nc.tensor.matmul(
                            ps_m[:, j, :],
                            lhsT=e_i[ts(0, P), ts(j, P)],
                            rhs=k_i,
                            start=(g == 0 and t == 0),
                            stop=(g == G - 1 and t == T - 1),
                        )

            # ---- normalize: mt = M rows (bf16), cs_j = rowsum -> s = 1/cs,
            # v = diag(s) W^T.  Chunks alternate DVE / ACT.
            sjs = []
            for j in range(DC):
                cs = stats.tile([P, 1], F32, tag="cs")
                if j % 2 == 0:
                    nc.vector.tensor_scalar(
                        out=mt_all[:, j, :], in0=ps_m[:, j, :],
                        scalar1=1.0, scalar2=0.0,
                        op0=mybir.AluOpType.mult,
                        op1=mybir.AluOpType.add,
                        accum_out=cs,
                    )
                else:
                    nc.scalar.activation(
                        out=mt_all[:, j, :], in_=ps_m[:, j, :],
                        func=mybir.ActivationFunctionType.Copy,
                        accum_out=cs,
                    )
                sj = stats.tile([P, 1], F32, tag="sj")
                nc.vector.reciprocal(out=sj, in_=cs)
                sjs.append(sj)
            for j in range(DC):
                if j % 2 == 0:
                    nc.vector.tensor_scalar_mul(
                        v_all[:, j, :], wt_all[:, j, :], sjs[j])
                else:
                    nc.scalar.mul(v_all[:, j, :], wt_all[:, j, :], sjs[j])

            # ---- C = M^T diag(s) W^T  ([D, D], bf16 chain)
            for k in range(DC):
                pc = psO.tile([P, D], F32, tag="po")
                for j in range(DC):
                    nc.tensor.matmul(
                        pc,
                        lhsT=mt_all[:, j, ts(k, P)],
                        rhs=v_all[:, j, :],
                        start=(j == 0), stop=(j == DC - 1),
                    )
                nc.scalar.copy(c_all[:, k, :], pc)

            # ---- phase 2: cast x -> bf16 (ACT), transpose 128-col blocks
            # (PE, bf16), out matmuls, bias on DVE, one 4KB-contiguous
            # output DMA per row group on the sync queue.
            for g in range(G):
                x_s = x_tiles[g]
                xb = xbp.tile([P, T, D], BF16, tag="xb")
                nc.scalar.copy(xb, x_s)
                og = ogp.tile([P, T, D], F32, tag="og")
                for t in range(T):
                    pt = psT.tile([P, DC, P], BF16, tag="pt")
                    for j in range(DC):
                        nc.tensor.transpose(
                            pt[:, j, :], xb[:, t, ts(j, P)], identb)
                    xt = xttp.tile([P, DC, P], BF16, tag="xt")
                    if t == 0:
                        nc.vector.tensor_copy(xt, pt)
                    else:
                        nc.scalar.copy(xt, pt)
                    po = psO.tile([P, D], F32, tag="po")
                    for j in range(DC):
                        nc.tensor.matmul(
                            po,
                            lhsT=xt[:, j, :],
                            rhs=c_all[:, j, :],
                            start=(j == 0), stop=(j == DC - 1),
                        )
                    nc.vector.tensor_add(og[:, t, :], po, bias_bc)
                nc.sync.dma_start(out=out_t[g], in_=og)

    nc.compile()
    return nc


def get_nc():
    if "nc" not in _CACHE:
        _CACHE["nc"] = _build_nc()
    return _CACHE["nc"]


def kernel(x, x2, W, b, _trace=False):
    nc = get_nc()
    in_maps = [
        {
            "x": np.ascontiguousarray(x[i], dtype=np.float32),
            "x2": np.ascontiguousarray(x2[i], dtype=np.float32),
            "W": np.ascontiguousarray(W, dtype=np.float32),
            "b": np.ascontiguousarray(b, dtype=np.float32),
        }
        for i in range(B)
    ]
    res = run_bass_kernel_spmd(nc, in_maps, list(range(B)), trace=_trace)
    out = np.stack([res.results[i]["out"] for i in range(B)], axis=0)
    if _trace:
        _CACHE["last_results"] = res
    return out
